# revision 1
# baseline (speedup 1.0000x reference)
import numpy as np
import jax
import jax.numpy as jnp
from functools import partial

H = 16
G = 4
MAX_POS = 128


def _attn_one_batch(x, Wq, Wk, Wv, Wo, E):
    # x: [T, D] for a single batch element
    T, D = x.shape
    hd = D // H
    r = H // G

    q = (x @ Wq.T).reshape(T, H, hd).transpose(1, 0, 2)  # [H,T,hd]
    k = (x @ Wk.T).reshape(T, G, hd)
    v = (x @ Wv.T).reshape(T, G, hd)

    k = jnp.repeat(k, r, axis=1).transpose(1, 0, 2)  # [H,T,hd]
    v = jnp.repeat(v, r, axis=1).transpose(1, 0, 2)  # [H,T,hd]

    q = q * (1.0 / hd) ** 0.5
    scores = jnp.einsum("hqd,hkd->hqk", q, k)  # [H,T,T]

    pos = jnp.arange(T)
    dist = jnp.clip(pos[None, :] - pos[:, None], -MAX_POS + 1, MAX_POS - 1) + MAX_POS - 1
    R = E[dist]  # [T,T,hd]  (R[kpos, qpos, d])
    bias = jnp.einsum("hqd,kqd->hqk", q, R)
    scores = scores + bias

    attn = jax.nn.softmax(scores, axis=-1)
    out = jnp.einsum("hqk,hkd->hqd", attn, v)  # [H,T,hd]
    out = out.transpose(1, 0, 2).reshape(T, D)
    return out @ Wo.T


def kernel(x, Wq, Wk, Wv, Wo, E):
    x = np.asarray(x, np.float32)
    Wq = np.asarray(Wq, np.float32)
    Wk = np.asarray(Wk, np.float32)
    Wv = np.asarray(Wv, np.float32)
    Wo = np.asarray(Wo, np.float32)
    E = np.asarray(E, np.float32)
    B = x.shape[0]
    try:
        devs = jax.devices()
        n = min(len(devs), B)
        if B % n != 0:
            n = 1
        # data-parallel over batch across the NeuronCores
        fn = jax.pmap(_attn_one_batch,
                      in_axes=(0, None, None, None, None, None),
                      devices=devs[:n]) if n > 1 else None
        if n > 1:
            per = B // n
            outs = []
            for i in range(per):
                xs = x[i * n:(i + 1) * n]
                outs.append(np.asarray(fn(xs, Wq, Wk, Wv, Wo, E)))
            return np.concatenate(outs, axis=0).astype(np.float32)
    except Exception:
        pass
    # fallback: single-device loop
    f = jax.jit(_attn_one_batch)
    return np.stack([np.asarray(f(x[b], Wq, Wk, Wv, Wo, E)) for b in range(B)]).astype(np.float32)

